# revision 15
# baseline (speedup 1.0000x reference)
"""Trainium2 Bass kernel for nn_AttentionAggregator (gnn_message_passing).

Two SPMD launches over 8 NeuronCores, data-parallel over nodes (512 users +
512 items per core), with a tiny host relay between them.

Structure (launch 1 is gather-descriptor-bound; everything else hides in
that shadow):
  - Algebraic reorder: relu(softmax(Q K^T) @ C @ W) == relu(softmax(Q K^T) @ (C @ W)).
  - k1 gathers embedding rows with SWDGE dma_gather.  The rate wall is
    ~2.3ns per descriptor (HBM-latency-bound SDMA service), so descriptors
    are minimized to one 256B element per gathered row: rows are fp16,
    zero-padded 64 -> 128 values.  Review rows come from a per-core
    COMPACTED table (the <=8192 distinct rows this core references per
    side) so indices fit int16 with no block/select tricks.
  - Under the gather shadow (emitted high-priority so the scheduler pins
    it at the head of the PE/Scalar queues): S^T = K q^T (fp16 PE matmuls),
    exp on ScalarE, E^T (bf16, 8MB) streamed to DRAM.  As gather chunks
    land: DVE repack (strip pad, pair two slots), PE transpose, DVE copy
    out of PSUM, and one N=512 C@W matmul per slot-pair accumulating h^T.
  - k2 only streams E^T + h back in and runs the PV matmuls
    (att[q, hid+1] accumulated over 32 m-tiles, ones column fused for
    the softmax denominator), then reciprocal-normalize + relu.
"""

import sys

for _p in ("/opt/trn_rl_repo",):
    if _p not in sys.path:
        sys.path.append(_p)

import numpy as np

import concourse.bacc as bacc
import concourse.mybir as mybir
import concourse.tile as tile
from concourse.bass_utils import run_bass_kernel_spmd
from concourse.masks import make_identity

F32 = mybir.dt.float32
BF16 = mybir.dt.bfloat16
FP16 = mybir.dt.float16
I16 = mybir.dt.int16
AF = mybir.ActivationFunctionType

N_REV, NU, DEG, D, HID = 100000, 4096, 16, 64, 128
N_CORES = 8
UB = NU // N_CORES          # 512 rows per core per side
NG = UB * DEG               # 8192 gathered entries per table per side
MT = NU // 128              # 32 m tiles
QT = UB // 128              # 4 q tiles
G = 2                       # m-tiles per QK/exp group
PADD = 2 * D                # rows padded to 128 fp16 = 256B (min gather elem)
NCH = 8                     # gather calls per (side, table); call c = slots 2c,2c+1
CH = NG // NCH              # 1024 entries per gather call
SCALE = 1.0 / float(np.sqrt(D))


def _build_k1():
    nc = bacc.Bacc("TRN2", target_bir_lowering=False, debug=False,
                   enable_asserts=True, num_devices=N_CORES,
                   num_swdge_queues=4)
    rtab = nc.dram_tensor("rtab", [2, NG, PADD], FP16, kind="ExternalInput")
    itab = nc.dram_tensor("itab", [2, NU, PADD], FP16, kind="ExternalInput")
    wrevp = nc.dram_tensor("wrevp", [128, 2, NCH, HID], BF16, kind="ExternalInput")
    witmp = nc.dram_tensor("witmp", [128, 2, NCH, HID], BF16, kind="ExternalInput")
    ridx = nc.dram_tensor("ridx", [128, 2, NG // 16], I16, kind="ExternalInput")
    iidx = nc.dram_tensor("iidx", [128, 2, NG // 16], I16, kind="ExternalInput")
    vt = nc.dram_tensor("vt", [D, 2, NU], FP16, kind="ExternalInput")
    vtq = nc.dram_tensor("vtq", [D, 2, UB], FP16, kind="ExternalInput")
    ht = nc.dram_tensor("ht", [2, HID, UB], BF16, kind="ExternalOutput")
    et = nc.dram_tensor("et", [2, MT // G, 128, G * UB], BF16, kind="ExternalOutput")

    with tile.TileContext(nc) as tc:
        with (
            tc.tile_pool(name="singles", bufs=1) as singles,
            tc.tile_pool(name="gp", bufs=2) as gp,
            tc.tile_pool(name="xtp", bufs=8) as xtp,
            tc.tile_pool(name="etp", bufs=3) as etp,
            tc.tile_pool(name="outb", bufs=2) as outb,
            tc.tile_pool(name="sps", bufs=3, space="PSUM") as sps,
            tc.tile_pool(name="tps", bufs=1, space="PSUM") as tps,
            tc.tile_pool(name="hps", bufs=1, space="PSUM") as hps,
        ):
            with tc.high_priority():
                ridx_sb = singles.tile([128, 2, NG // 16], I16)
                nc.sync.dma_start(out=ridx_sb[:], in_=ridx.ap())
                iidx_sb = singles.tile([128, 2, NG // 16], I16)
                nc.sync.dma_start(out=iidx_sb[:], in_=iidx.ap())
                vtq_sb = singles.tile([D, 2, UB], FP16)
                nc.sync.dma_start(out=vtq_sb[:], in_=vtq.ap())
                vt_sb = singles.tile([D, 2, NU], FP16)
                nc.sync.dma_start(out=vt_sb[:], in_=vt.ap())
            wrev_sb = singles.tile([128, 2, NCH, HID], BF16)
            nc.sync.dma_start(out=wrev_sb[:], in_=wrevp.ap())
            witm_sb = singles.tile([128, 2, NCH, HID], BF16)
            nc.sync.dma_start(out=witm_sb[:], in_=witmp.ap())
            identf = singles.tile([128, 128], FP16, tag="identf")
            make_identity(nc, identf[:])

            # ---- QK + exp + E^T out: high priority so PE/ScalarE start on
            # this stream at t=0 while gathers drain on SWDGE ----
            etb_last = None
            with tc.high_priority():
                for side in range(2):
                    for g in range(MT // G):
                        s_ps = sps.tile([128, G, UB], F32, tag="sps", name=f"s{side}_{g}")
                        for k in range(G):
                            m = g * G + k
                            nc.tensor.matmul(
                                s_ps[:, k, :],
                                lhsT=vt_sb[:, side, m * 128:(m + 1) * 128],
                                rhs=vtq_sb[:, side, :],
                                start=True, stop=True, skip_group_check=True)
                        etb = etp.tile([128, G, UB], BF16, tag="etb", name=f"e{side}_{g}")
                        nc.scalar.activation(etb[:], s_ps[:], AF.Exp, scale=SCALE)
                        nc.sync.dma_start(out=et.ap()[side, g], in_=etb[:])
                        etb_last = etb
                # all-ones mask derived from the last exp tile (exp > 0): a pure
                # ordering device — everything predicated on it is pinned after
                # the QK/exp phase on its engine queue, so the scheduler cannot
                # head-of-line block the shadow work with gather-stalled ops.
                maskp = singles.tile([128, UB], mybir.dt.int8, tag="maskp")
                nc.vector.tensor_scalar(out=maskp[:], in0=etb_last[:, 0, :],
                                        scalar1=0.0, scalar2=None,
                                        op0=mybir.AluOpType.is_ge)

            # ---- all gather calls: pure stream on the GPSIMD queue ----
            dsts = {}
            qctr = 0
            for side in range(2):
                for tbl, (tabd, idx_sb) in enumerate(((rtab, ridx_sb), (itab, iidx_sb))):
                    for c in range(NCH):
                        dst = gp.tile([128, CH // 128, PADD], FP16,
                                      tag=f"g{tbl}{c}", name=f"g{side}_{tbl}_{c}")
                        nc.gpsimd.dma_gather(
                            out_ap=dst[:],
                            in_ap=tabd.ap()[side],
                            idxs_ap=idx_sb[:, side, c * (CH // 16):(c + 1) * (CH // 16)],
                            num_idxs=CH, num_idxs_reg=CH, elem_size=PADD,
                            transpose=False, single_packet=False,
                            queue_num=qctr % 4,
                        )
                        qctr += 1
                        dsts[(side, tbl, c)] = dst

            # ---- repack + transpose + C@W as gather chunks land ----
            for side in range(2):
                h_ps = hps.tile([HID, UB], F32, tag="hps", name=f"hps_{side}")
                for tbl, w_sb in enumerate((wrev_sb, witm_sb)):
                    for c in range(NCH):
                        dst = dsts[(side, tbl, c)]
                        xt = xtp.tile([128, 4, 128], FP16, tag="xt",
                                      name=f"x{side}_{tbl}_{c}")
                        # one repack per chunk: strip the pad and interleave the
                        # two slots per group so each transpose input is a
                        # contiguous [128, 2, 64] run.  Done as a predicated
                        # copy masked by the last exp tile (exp > 0 always, so
                        # every element copies): the mask is a pure ordering
                        # device that pins the whole chunk-compute chain after
                        # the QK/exp phase on the PE/DVE queues — otherwise the
                        # scheduler interleaves gather-stalled transposes ahead
                        # of QK and head-of-line blocks the shadow work.
                        xr = xtp.tile([128, 4, 2, D], FP16, tag="xr",
                                      name=f"r{side}_{tbl}_{c}")
                        nc.vector.copy_predicated(
                            xr[:],
                            maskp[:].rearrange("p (g h d) -> p g h d", g=4, h=2),
                            dst[:, :, 0:D].rearrange("p (h g) d -> p g h d", h=2))
                        for g in range(4):
                            t_ps = tps.tile([128, 128], FP16, tag="tps",
                                            name=f"t{side}_{tbl}_{c}_{g}")
                            nc.tensor.transpose(t_ps[:], xr[:, g, :, :], identf[:])
                            nc.vector.tensor_copy(xt[:, g, :], t_ps[:])
                        nc.tensor.matmul(
                            h_ps[:],
                            lhsT=w_sb[:, side, c, :],
                            rhs=xt[:],
                            start=(tbl == 0 and c == 0),
                            stop=(tbl == 1 and c == NCH - 1),
                            skip_group_check=True)
                h_sb = outb.tile([HID, UB], BF16, tag="hsb", name=f"hsb_{side}")
                nc.vector.tensor_copy(h_sb[:], h_ps[:])
                nc.sync.dma_start(out=ht.ap()[side], in_=h_sb[:])

    nc.compile()
    return nc


def _build_k2():
    nc = bacc.Bacc("TRN2", target_bir_lowering=False, debug=False,
                   enable_asserts=True, num_devices=N_CORES)
    et = nc.dram_tensor("et", [2, MT // G, 128, G * UB], BF16, kind="ExternalInput")
    ha = nc.dram_tensor("ha", [2, 128, MT, HID + 1], BF16, kind="ExternalInput")
    uo = nc.dram_tensor("uo", [UB, HID], F32, kind="ExternalOutput")
    io = nc.dram_tensor("io", [UB, HID], F32, kind="ExternalOutput")

    GC = 2   # exp-groups (2 m-tiles each) per E^T load
    with tile.TileContext(nc) as tc:
        with (
            tc.tile_pool(name="hap", bufs=2) as hap,
            tc.tile_pool(name="etp", bufs=4) as etp,
            tc.tile_pool(name="ob", bufs=4) as obp,
            tc.tile_pool(name="aps", bufs=1, space="PSUM") as aps,
        ):
            for side, out_d in enumerate((uo, io)):
                ha_sb = hap.tile([128, MT, HID + 1], BF16, tag="ha", name=f"ha_{side}")
                nc.sync.dma_start(out=ha_sb[:], in_=ha.ap()[side])

                att_ps = [aps.tile([128, HID + 1], F32, tag=f"att{qt}",
                                   name=f"att{qt}_{side}")
                          for qt in range(QT)]
                for gc in range(MT // G // GC):
                    et_sb = etp.tile([128, GC, G * UB], BF16, tag="et",
                                     name=f"et_{side}_{gc}")
                    nc.sync.dma_start(
                        out=et_sb[:],
                        in_=et.ap()[side, gc * GC:(gc + 1) * GC].rearrange("g p q -> p g q"))
                    for g in range(GC):
                        for k in range(G):
                            m = (gc * GC + g) * G + k
                            for qt in range(QT):
                                nc.tensor.matmul(
                                    att_ps[qt][:],
                                    lhsT=et_sb[:, g, k * UB + qt * 128:k * UB + (qt + 1) * 128],
                                    rhs=ha_sb[:, m, :],
                                    start=(m == 0), stop=(m == MT - 1),
                                    skip_group_check=True)
                for qt in range(QT):
                    recip = obp.tile([128, 1], F32, tag="recip")
                    nc.vector.reciprocal(recip[:], att_ps[qt][:, HID:HID + 1])
                    o_sb = obp.tile([128, HID], F32, tag="osb")
                    nc.scalar.activation(o_sb[:], att_ps[qt][:, 0:HID], AF.Relu,
                                         scale=recip[:, 0:1])
                    nc.sync.dma_start(out=out_d.ap()[qt * 128:(qt + 1) * 128, :],
                                      in_=o_sb[:])
    nc.compile()
    return nc


_CACHE = {}


def _programs():
    if "k1" not in _CACHE:
        _CACHE["k1"] = _build_k1()
        _CACHE["k2"] = _build_k2()
    return _CACHE["k1"], _CACHE["k2"]


def _wrap16(a):
    # flat int list -> [128, n/16] int16: index i at partition i%16, slot
    # i//16, replicated for the 8 Q7 cores
    a = np.asarray(a)
    return np.tile(a.reshape(-1, 16).T, (8, 1)).astype(np.int16)


def _pad_rows(tab16):
    n = tab16.shape[0]
    out = np.zeros((n, PADD), dtype=tab16.dtype)
    out[:, :D] = tab16
    return out


def kernel(review_vecs, user_vecs, item_vecs, user_weights, item_weights,
           adj0, adj1, adj2, adj3, _profile=None):
    import ml_dtypes

    rev = np.asarray(review_vecs, np.float32)
    uv = np.asarray(user_vecs, np.float32)
    iv = np.asarray(item_vecs, np.float32)
    wu = np.asarray(user_weights, np.float32)
    wi = np.asarray(item_weights, np.float32)
    a0, a1, a2, a3 = (np.asarray(a).astype(np.int64) for a in (adj0, adj1, adj2, adj3))

    rev16 = rev.astype(np.float16)
    itab = np.ascontiguousarray(np.stack([_pad_rows(iv.astype(np.float16)),
                                          _pad_rows(uv.astype(np.float16))]))

    # slot-pair stacked weight blocks, pre-transposed to SBUF layout
    # [128 rows, side, pair, hid]: pair c rows = [W_2c (64) ; W_2c+1 (64)]
    def _wpairs(w):
        w4 = w.reshape(DEG, 2, D, HID)          # [j, rev/itm, d, hid]
        wr = w4[:, 0].reshape(NCH, 2 * D, HID).transpose(1, 0, 2)  # [128, pair, hid]
        wo = w4[:, 1].reshape(NCH, 2 * D, HID).transpose(1, 0, 2)
        return wr, wo
    wr_u, wi_u = _wpairs(wu)
    wr_i, wi_i = _wpairs(wi)
    wrevp = np.ascontiguousarray(
        np.stack([wr_u, wr_i], axis=1).astype(ml_dtypes.bfloat16))   # [128,2,8,128]
    witmp = np.ascontiguousarray(
        np.stack([wi_u, wi_i], axis=1).astype(ml_dtypes.bfloat16))

    vtf = np.ascontiguousarray(
        np.stack([uv.T, iv.T], axis=1).astype(np.float16))           # [64,2,4096]

    k1, k2 = _programs()
    cores = list(range(N_CORES))

    in_maps1 = []
    for c in cores:
        rtabc = np.zeros((2, NG, PADD), np.float16)
        ridx = np.zeros((128, 2, NG // 16), np.int16)
        iidx = np.zeros((128, 2, NG // 16), np.int16)
        for side, (a_rev, a_oth) in enumerate(((a0, a1), (a2, a3))):
            blk_r = a_rev[c * UB:(c + 1) * UB]          # [UB, DEG]
            blk_o = a_oth[c * UB:(c + 1) * UB]
            ent_r = blk_r.T.reshape(-1)                  # slot-major: e = j*UB + r
            ent_o = blk_o.T.reshape(-1)
            uniq, inv = np.unique(ent_r, return_inverse=True)
            rtabc[side, :len(uniq), :D] = rev16[uniq]
            ridx[:, side, :] = _wrap16(inv)
            iidx[:, side, :] = _wrap16(ent_o)
        in_maps1.append({
            "rtab": rtabc, "itab": itab, "wrevp": wrevp, "witmp": witmp,
            "ridx": ridx, "iidx": iidx,
            "vt": vtf,
            "vtq": np.ascontiguousarray(vtf[:, :, c * UB:(c + 1) * UB]),
        })
    r1 = run_bass_kernel_spmd(k1, in_maps1, core_ids=cores, trace=_profile is not None)

    # host relay: assemble full h (+ones col), tile for k2's PV
    ha = np.ones((2, NU, HID + 1), np.float32)
    for side in range(2):
        ha[side, :, :HID] = np.concatenate(
            [np.asarray(r1.results[c]["ht"][side], np.float32).T for c in cores], axis=0)
    ha = ha.reshape(2, MT, 128, HID + 1).transpose(0, 2, 1, 3)
    ha = np.ascontiguousarray(ha.astype(ml_dtypes.bfloat16))

    in_maps2 = [{"et": r1.results[c]["et"], "ha": ha} for c in cores]
    r2 = run_bass_kernel_spmd(k2, in_maps2, core_ids=cores, trace=_profile is not None)

    user_out = np.concatenate([r2.results[c]["uo"] for c in cores], axis=0)
    item_out = np.concatenate([r2.results[c]["io"] for c in cores], axis=0)

    if _profile is not None:
        _profile["k1"] = r1
        _profile["k2"] = r2
    return user_out, item_out


# revision 18
# speedup vs baseline: 1.3693x; 1.3693x over previous
"""Trainium2 Bass kernel for nn_AttentionAggregator (gnn_message_passing).

Two SPMD launches over 8 NeuronCores, data-parallel over nodes (512 users +
512 items per core), with a tiny host relay between them.

Structure (launch 1 is gather-descriptor-bound; everything else hides in
that shadow):
  - Algebraic reorder: relu(softmax(Q K^T) @ C @ W) == relu(softmax(Q K^T) @ (C @ W)).
  - k1 gathers embedding rows with SWDGE dma_gather.  The rate wall is
    ~2.3ns per descriptor (HBM-latency-bound SDMA service), so descriptors
    are minimized to one 256B element per gathered row: rows are fp16,
    zero-padded 64 -> 128 values.  Review rows come from a per-core
    COMPACTED table (the <=8192 distinct rows this core references per
    side) so indices fit int16 with no block/select tricks.
  - Under the gather shadow (emitted high-priority so the scheduler pins
    it at the head of the PE/Scalar queues): S^T = K q^T (fp16 PE matmuls),
    exp on ScalarE, E^T (bf16, 8MB) streamed to DRAM.  As gather chunks
    land: DVE repack (strip pad, pair two slots), PE transpose, DVE copy
    out of PSUM, and one N=512 C@W matmul per slot-pair accumulating h^T.
  - k2 only streams E^T + h back in and runs the PV matmuls
    (att[q, hid+1] accumulated over 32 m-tiles, ones column fused for
    the softmax denominator), then reciprocal-normalize + relu.
"""

import sys

for _p in ("/opt/trn_rl_repo",):
    if _p not in sys.path:
        sys.path.append(_p)

import numpy as np

import concourse.bacc as bacc
import concourse.mybir as mybir
import concourse.tile as tile
from concourse.bass_utils import run_bass_kernel_spmd
from concourse.masks import make_identity

F32 = mybir.dt.float32
BF16 = mybir.dt.bfloat16
FP16 = mybir.dt.float16
I16 = mybir.dt.int16
AF = mybir.ActivationFunctionType

N_REV, NU, DEG, D, HID = 100000, 4096, 16, 64, 128
N_CORES = 8
UB = NU // N_CORES          # 512 rows per core per side
NG = UB * DEG               # 8192 gathered entries per table per side
MT = NU // 128              # 32 m tiles
QT = UB // 128              # 4 q tiles
G = 2                       # m-tiles per QK/exp group
PADD = 2 * D                # rows padded to 128 fp16 = 256B (min gather elem)
NCH = 8                     # gather calls per (side, table); call c = slots 2c,2c+1
CH = NG // NCH              # 1024 entries per gather call
SCALE = 1.0 / float(np.sqrt(D))


def _build_k1():
    nc = bacc.Bacc("TRN2", target_bir_lowering=False, debug=False,
                   enable_asserts=True, num_devices=N_CORES,
                   num_swdge_queues=4)
    rtab = nc.dram_tensor("rtab", [2, NG, PADD], FP16, kind="ExternalInput")
    itab = nc.dram_tensor("itab", [2, NU, PADD], FP16, kind="ExternalInput")
    wrevp = nc.dram_tensor("wrevp", [128, 2, NCH, HID], BF16, kind="ExternalInput")
    witmp = nc.dram_tensor("witmp", [128, 2, NCH, HID], BF16, kind="ExternalInput")
    ridx = nc.dram_tensor("ridx", [128, 2, NG // 16], I16, kind="ExternalInput")
    iidx = nc.dram_tensor("iidx", [128, 2, NG // 16], I16, kind="ExternalInput")
    vt = nc.dram_tensor("vt", [D, 2, NU], FP16, kind="ExternalInput")
    vtq = nc.dram_tensor("vtq", [D, 2, UB], FP16, kind="ExternalInput")
    ht = nc.dram_tensor("ht", [2, HID, UB], BF16, kind="ExternalOutput")
    et = nc.dram_tensor("et", [2, MT // G, 128, G * UB], BF16, kind="ExternalOutput")

    with tile.TileContext(nc) as tc:
        with (
            tc.tile_pool(name="singles", bufs=1) as singles,
            tc.tile_pool(name="gp", bufs=2) as gp,
            tc.tile_pool(name="xtp", bufs=8) as xtp,
            tc.tile_pool(name="etp", bufs=3) as etp,
            tc.tile_pool(name="outb", bufs=2) as outb,
            tc.tile_pool(name="sps", bufs=2, space="PSUM") as sps,
            tc.tile_pool(name="tps", bufs=2, space="PSUM") as tps,
            tc.tile_pool(name="hps", bufs=1, space="PSUM") as hps,
        ):
            with tc.high_priority():
                # priming DMA: absorbs the ~15us first-DMA init latency so the
                # idx loads (and thus the gathers) start early
                prime = singles.tile([1, 64], FP16, tag="prime")
                nc.sync.dma_start(out=prime[:], in_=itab.ap()[0, 0:1, 0:64])
                ridx_sb = singles.tile([128, 2, NG // 16], I16)
                nc.sync.dma_start(out=ridx_sb[:], in_=ridx.ap())
                iidx_sb = singles.tile([128, 2, NG // 16], I16)
                nc.sync.dma_start(out=iidx_sb[:], in_=iidx.ap())
                vtq_sb = singles.tile([D, 2, UB], FP16)
                nc.sync.dma_start(out=vtq_sb[:], in_=vtq.ap())
                vt_sb = singles.tile([D, 2, NU], FP16)
                nc.sync.dma_start(out=vt_sb[:], in_=vt.ap())
            wrev_sb = singles.tile([128, 2, NCH, HID], BF16)
            nc.sync.dma_start(out=wrev_sb[:], in_=wrevp.ap())
            witm_sb = singles.tile([128, 2, NCH, HID], BF16)
            nc.sync.dma_start(out=witm_sb[:], in_=witmp.ap())
            identf = singles.tile([128, 128], FP16, tag="identf")
            make_identity(nc, identf[:])

            # ---- QK + exp + E^T out: high priority so PE/ScalarE start on
            # this stream at t=0 while gathers drain on SWDGE ----
            masks = []
            with tc.high_priority():
                for side in range(2):
                    for g in range(MT // G):
                        s_ps = sps.tile([128, G, UB], F32, tag="sps", name=f"s{side}_{g}")
                        for k in range(G):
                            m = g * G + k
                            nc.tensor.matmul(
                                s_ps[:, k, :],
                                lhsT=vt_sb[:, side, m * 128:(m + 1) * 128],
                                rhs=vtq_sb[:, side, :],
                                start=True, stop=True, skip_group_check=True)
                        etb = etp.tile([128, G, UB], BF16, tag="etb", name=f"e{side}_{g}")
                        nc.scalar.activation(etb[:], s_ps[:], AF.Exp, scale=SCALE)
                        nc.sync.dma_start(out=et.ap()[side, g], in_=etb[:])
                    # all-ones mask derived from this side's last exp tile
                    # (exp > 0 always): a pure ordering device — repacks
                    # predicated on it are pinned after this side's QK/exp on
                    # the DVE queue, so the scheduler cannot head-of-line
                    # block the shadow work with gather-stalled chunk ops.
                    maskp = singles.tile([128, UB], mybir.dt.int8,
                                         tag=f"maskp{side}", name=f"maskp{side}")
                    nc.vector.tensor_scalar(out=maskp[:], in0=etb[:, 0, :],
                                            scalar1=0.0, scalar2=None,
                                            op0=mybir.AluOpType.is_ge)
                    masks.append(maskp)

            # ---- all gather calls: pure stream on the GPSIMD queue ----
            dsts = {}
            qctr = 0
            for side in range(2):
                for tbl, (tabd, idx_sb) in enumerate(((rtab, ridx_sb), (itab, iidx_sb))):
                    for c in range(NCH):
                        dst = gp.tile([128, CH // 128, PADD], FP16,
                                      tag=f"g{tbl}{c}", name=f"g{side}_{tbl}_{c}")
                        nc.gpsimd.dma_gather(
                            out_ap=dst[:],
                            in_ap=tabd.ap()[side],
                            idxs_ap=idx_sb[:, side, c * (CH // 16):(c + 1) * (CH // 16)],
                            num_idxs=CH, num_idxs_reg=CH, elem_size=PADD,
                            transpose=False, single_packet=False,
                            queue_num=qctr % 4,
                        )
                        qctr += 1
                        dsts[(side, tbl, c)] = dst

            # ---- repack + transpose + C@W as gather chunks land ----
            for side in range(2):
                h_ps = hps.tile([HID, UB], F32, tag="hps", name=f"hps_{side}")
                for tbl, w_sb in enumerate((wrev_sb, witm_sb)):
                    for c in range(NCH):
                        dst = dsts[(side, tbl, c)]
                        xt = xtp.tile([128, 4, 128], FP16, tag="xt",
                                      name=f"x{side}_{tbl}_{c}")
                        # one repack per chunk: strip the pad and interleave the
                        # two slots per group so each transpose input is a
                        # contiguous [128, 2, 64] run.  Done as a predicated
                        # copy masked by the last exp tile (exp > 0 always, so
                        # every element copies): the mask is a pure ordering
                        # device that pins the whole chunk-compute chain after
                        # the QK/exp phase on the PE/DVE queues — otherwise the
                        # scheduler interleaves gather-stalled transposes ahead
                        # of QK and head-of-line blocks the shadow work.
                        xr = xtp.tile([128, 4, 2, D], FP16, tag="xr",
                                      name=f"r{side}_{tbl}_{c}")
                        nc.vector.copy_predicated(
                            xr[:],
                            masks[side][:].rearrange("p (g h d) -> p g h d", g=4, h=2),
                            dst[:, :, 0:D].rearrange("p (h g) d -> p g h d", h=2))
                        for gp2 in range(2):
                            t_ps = tps.tile([128, 2, 128], FP16, tag="tps",
                                            name=f"t{side}_{tbl}_{c}_{gp2}")
                            for gg in range(2):
                                nc.tensor.transpose(
                                    t_ps[:, gg, :], xr[:, gp2 * 2 + gg, :, :],
                                    identf[:])
                            nc.vector.tensor_copy(
                                xt[:, gp2 * 2:(gp2 + 1) * 2, :], t_ps[:])
                        nc.tensor.matmul(
                            h_ps[:],
                            lhsT=w_sb[:, side, c, :],
                            rhs=xt[:],
                            start=(tbl == 0 and c == 0),
                            stop=(tbl == 1 and c == NCH - 1),
                            skip_group_check=True)
                h_sb = outb.tile([HID, UB], BF16, tag="hsb", name=f"hsb_{side}")
                nc.vector.tensor_copy(h_sb[:], h_ps[:])
                nc.sync.dma_start(out=ht.ap()[side], in_=h_sb[:])

    nc.compile()
    return nc


def _build_k2():
    nc = bacc.Bacc("TRN2", target_bir_lowering=False, debug=False,
                   enable_asserts=True, num_devices=N_CORES)
    et = nc.dram_tensor("et", [2, MT // G, 128, G * UB], BF16, kind="ExternalInput")
    ha = nc.dram_tensor("ha", [2, 128, MT, HID + 1], BF16, kind="ExternalInput")
    uo = nc.dram_tensor("uo", [UB, HID], F32, kind="ExternalOutput")
    io = nc.dram_tensor("io", [UB, HID], F32, kind="ExternalOutput")

    GC = 2   # exp-groups (2 m-tiles each) per E^T load
    with tile.TileContext(nc) as tc:
        with (
            tc.tile_pool(name="hap", bufs=2) as hap,
            tc.tile_pool(name="etp", bufs=4) as etp,
            tc.tile_pool(name="ob", bufs=4) as obp,
            tc.tile_pool(name="aps", bufs=1, space="PSUM") as aps,
        ):
            for side, out_d in enumerate((uo, io)):
                ha_sb = hap.tile([128, MT, HID + 1], BF16, tag="ha", name=f"ha_{side}")
                nc.sync.dma_start(out=ha_sb[:], in_=ha.ap()[side])

                att_ps = [aps.tile([128, HID + 1], F32, tag=f"att{qt}",
                                   name=f"att{qt}_{side}")
                          for qt in range(QT)]
                for gc in range(MT // G // GC):
                    et_sb = etp.tile([128, GC, G * UB], BF16, tag="et",
                                     name=f"et_{side}_{gc}")
                    nc.sync.dma_start(
                        out=et_sb[:],
                        in_=et.ap()[side, gc * GC:(gc + 1) * GC].rearrange("g p q -> p g q"))
                    for g in range(GC):
                        for k in range(G):
                            m = (gc * GC + g) * G + k
                            for qt in range(QT):
                                nc.tensor.matmul(
                                    att_ps[qt][:],
                                    lhsT=et_sb[:, g, k * UB + qt * 128:k * UB + (qt + 1) * 128],
                                    rhs=ha_sb[:, m, :],
                                    start=(m == 0), stop=(m == MT - 1),
                                    skip_group_check=True)
                for qt in range(QT):
                    recip = obp.tile([128, 1], F32, tag="recip")
                    nc.vector.reciprocal(recip[:], att_ps[qt][:, HID:HID + 1])
                    o_sb = obp.tile([128, HID], F32, tag="osb")
                    nc.scalar.activation(o_sb[:], att_ps[qt][:, 0:HID], AF.Relu,
                                         scale=recip[:, 0:1])
                    nc.sync.dma_start(out=out_d.ap()[qt * 128:(qt + 1) * 128, :],
                                      in_=o_sb[:])
    nc.compile()
    return nc


_CACHE = {}


def _programs():
    if "k1" not in _CACHE:
        _CACHE["k1"] = _build_k1()
        _CACHE["k2"] = _build_k2()
    return _CACHE["k1"], _CACHE["k2"]


def _wrap16(a):
    # flat int list -> [128, n/16] int16: index i at partition i%16, slot
    # i//16, replicated for the 8 Q7 cores
    a = np.asarray(a)
    return np.tile(a.reshape(-1, 16).T, (8, 1)).astype(np.int16)


def _pad_rows(tab16):
    n = tab16.shape[0]
    out = np.zeros((n, PADD), dtype=tab16.dtype)
    out[:, :D] = tab16
    return out


def kernel(review_vecs, user_vecs, item_vecs, user_weights, item_weights,
           adj0, adj1, adj2, adj3, _profile=None):
    import ml_dtypes

    rev = np.asarray(review_vecs, np.float32)
    uv = np.asarray(user_vecs, np.float32)
    iv = np.asarray(item_vecs, np.float32)
    wu = np.asarray(user_weights, np.float32)
    wi = np.asarray(item_weights, np.float32)
    a0, a1, a2, a3 = (np.asarray(a).astype(np.int64) for a in (adj0, adj1, adj2, adj3))

    rev16 = rev.astype(np.float16)
    itab = np.ascontiguousarray(np.stack([_pad_rows(iv.astype(np.float16)),
                                          _pad_rows(uv.astype(np.float16))]))

    # slot-pair stacked weight blocks, pre-transposed to SBUF layout
    # [128 rows, side, pair, hid]: pair c rows = [W_2c (64) ; W_2c+1 (64)]
    def _wpairs(w):
        w4 = w.reshape(DEG, 2, D, HID)          # [j, rev/itm, d, hid]
        wr = w4[:, 0].reshape(NCH, 2 * D, HID).transpose(1, 0, 2)  # [128, pair, hid]
        wo = w4[:, 1].reshape(NCH, 2 * D, HID).transpose(1, 0, 2)
        return wr, wo
    wr_u, wi_u = _wpairs(wu)
    wr_i, wi_i = _wpairs(wi)
    wrevp = np.ascontiguousarray(
        np.stack([wr_u, wr_i], axis=1).astype(ml_dtypes.bfloat16))   # [128,2,8,128]
    witmp = np.ascontiguousarray(
        np.stack([wi_u, wi_i], axis=1).astype(ml_dtypes.bfloat16))

    vtf = np.ascontiguousarray(
        np.stack([uv.T, iv.T], axis=1).astype(np.float16))           # [64,2,4096]

    k1, k2 = _programs()
    cores = list(range(N_CORES))

    in_maps1 = []
    for c in cores:
        rtabc = np.zeros((2, NG, PADD), np.float16)
        ridx = np.zeros((128, 2, NG // 16), np.int16)
        iidx = np.zeros((128, 2, NG // 16), np.int16)
        for side, (a_rev, a_oth) in enumerate(((a0, a1), (a2, a3))):
            blk_r = a_rev[c * UB:(c + 1) * UB]          # [UB, DEG]
            blk_o = a_oth[c * UB:(c + 1) * UB]
            ent_r = blk_r.T.reshape(-1)                  # slot-major: e = j*UB + r
            ent_o = blk_o.T.reshape(-1)
            uniq, inv = np.unique(ent_r, return_inverse=True)
            rtabc[side, :len(uniq), :D] = rev16[uniq]
            ridx[:, side, :] = _wrap16(inv)
            iidx[:, side, :] = _wrap16(ent_o)
        in_maps1.append({
            "rtab": rtabc, "itab": itab, "wrevp": wrevp, "witmp": witmp,
            "ridx": ridx, "iidx": iidx,
            "vt": vtf,
            "vtq": np.ascontiguousarray(vtf[:, :, c * UB:(c + 1) * UB]),
        })
    r1 = run_bass_kernel_spmd(k1, in_maps1, core_ids=cores, trace=_profile is not None)

    # host relay: assemble full h (+ones col), tile for k2's PV
    ha = np.ones((2, NU, HID + 1), np.float32)
    for side in range(2):
        ha[side, :, :HID] = np.concatenate(
            [np.asarray(r1.results[c]["ht"][side], np.float32).T for c in cores], axis=0)
    ha = ha.reshape(2, MT, 128, HID + 1).transpose(0, 2, 1, 3)
    ha = np.ascontiguousarray(ha.astype(ml_dtypes.bfloat16))

    in_maps2 = [{"et": r1.results[c]["et"], "ha": ha} for c in cores]
    r2 = run_bass_kernel_spmd(k2, in_maps2, core_ids=cores, trace=_profile is not None)

    user_out = np.concatenate([r2.results[c]["uo"] for c in cores], axis=0)
    item_out = np.concatenate([r2.results[c]["io"] for c in cores], axis=0)

    if _profile is not None:
        _profile["k1"] = r1
        _profile["k2"] = r2
    return user_out, item_out
